# revision 84
# baseline (speedup 1.0000x reference)
"""Trainium2 Bass kernel for nn_Attention (dense_transformer).

Reference computation (per batch n of 4):
  qkv = W_qkv @ x + b          (384, 4096)   [x flattened to (256, 64*64)]
  raw C-order reinterpret of qkv flat buffer as (4096, 384) -> q|k|v (4096,128) each
  scores = q @ k.T / 64        (4096, 4096)
  soft = softmax(scores, axis=-2)             [column softmax]
  out = soft @ v               (4096, 128)
  raw reinterpret of out as (128, 4096)
  y = W_out @ out2 + b_out     (256, 4096)

Sharding: 8 cores = 4 batches x 2 column-chunks (j-axis of the score
matrix = rows of k/v).  Column-softmax stats are local to a j-chunk;
each core produces a partial y, host sums the pair.  The SPMD graph is
identical on all cores; the j-half selection is encoded host-side by
rotating the qkv output channels by 192 for odd cores and rotating
W_out's e-axis by 64 to compensate.

Compute layout (per core), final:
  head:    merged-const + x loads and F = W_qkv@x+b (bf16) -> fbuf
           half-tile writes interleaved with MERGED q|k xbar
           transposes (qk0/qk1/qT2) on the sync ring, with explicit
           sync=False ring-order hints (the DMA device is an exclusive
           serial resource; per-DMA dispatch latency ~2us).  F drains
           split 2 DVE + 2 ACT per half-tile.  v gather via SWDGE,
           ordered after the transposes (tail-only data).
  phase A: ACT runs ONLY exps (the critical path, ~68us).  Lead
           chunks (jb0-3 x 1024-wide halves of i<2048, qk0/qk1 only)
           start ~20us; then 28 x 2048-wide chunks (psA = 2 x 4 PSUM
           banks).  Each exp: 4 score MMs bf16 -> PSUM, one exp with
           accum_out -> bf16 ring (PR=8 slots, WAR-safe via deferred
           converts).
  stats:   DVE: zsum = reduce(zacc); szinv = 1/(zsum/4096) = 4096/Z.
  converts (DVE/Pool alternating, ACT for the tail-gating jb15-H0,
           emission deferred one chunk so DVE's FIFO never blocks the
           psA recycle): dPn8 = fp8e4(P*szinv - 1) scattered into P8
           so column i' = hb*128+e2 holds score row i = e2*32+hb.
           v8 = fp8e4(v) plain.
  tail:    colsum row cs[e] = sum_j v_bf16[j,e] (16 rank-reducing MMs
           to a (1,128) PSUM row; the rank-1 "1" term of Pn = 1+dPn);
           per 512-wide group g, per hb: K=1 seed MM (ones x cs_row)
           + 8 fp8 DoubleRow pair-MMs with P8 STATIONARY, which
           produce out2[e2, hb*128+e] DIRECTLY in PSUM (no TensorE
           transposes needed); drain ACT/DVE, proj2 (woutT pre-scaled
           by 1/4096 host-side) + b_out, y -> sync + gpsimd rings.
"""

import numpy as np
import ml_dtypes

import concourse.bass as bass
import concourse.bacc as bacc
import concourse.mybir as mybir
from concourse.bass_utils import run_bass_kernel_spmd
from concourse.tile import TileContext, add_dep_helper
from concourse.masks import make_identity

BF16 = mybir.dt.bfloat16
F32 = mybir.dt.float32
FP8 = mybir.dt.float8e4
AF = mybir.ActivationFunctionType
ALU = mybir.AluOpType
DR = mybir.MatmulPerfMode.DoubleRow

N, C, E, O, HW = 4, 256, 128, 384, 4096
JC = HW // 2          # j-chunk per core
NJB = JC // 128       # 16 j-blocks
SCALE = 1.0 / 64.0    # 1/sqrt(HW)
SFIX = 4096.0         # softmax renorm: Pn = P * (4096/Z), undone in W_out

_CACHE = {}


def build_nc():
    nc = bacc.Bacc("TRN2", target_bir_lowering=False, debug=False, num_devices=8)

    x_ext = nc.dram_tensor("x", [C, HW], BF16, kind="ExternalInput").ap()
    wqkvT_ext = nc.dram_tensor("wqkvT", [C, O], BF16, kind="ExternalInput").ap()
    bqkv_ext = nc.dram_tensor("bqkv", [O, 1], F32, kind="ExternalInput").ap()
    woutT_ext = nc.dram_tensor("woutT", [E, C], BF16, kind="ExternalInput").ap()
    bout_ext = nc.dram_tensor("bout", [C, 1], F32, kind="ExternalInput").ap()
    boutr_ext = nc.dram_tensor("boutr", [1, C], BF16, kind="ExternalInput").ap()
    y_ext = nc.dram_tensor("out", [C, HW], BF16, kind="ExternalOutput").ap()

    fbuf = nc.dram_tensor("fbuf", [O * HW], BF16).ap()
    fview_o = fbuf.rearrange("(o hw) -> o hw", hw=HW)   # (384, 4096) write view
    fview_i = fbuf.rearrange("(i j) -> i j", j=O)        # (4096, 384) read view

    # persistent SBUF.  qT/kT/v split at 512-aligned boundaries covered by
    # successive F o-tiles.
    # merged q|k transpose targets: qk0 = rows 0-1024 x cols 0-256 of the
    # (i, j) view (q part 0 | k part 0), qk1 = rows 1024-2560 (q part 1 |
    # k part 1 + 512 junk), qT2t = rows 2560-4096 x q only.
    qk0 = nc.alloc_sbuf_tensor("qk0", [128, 2048], BF16).ap()
    qk1 = nc.alloc_sbuf_tensor("qk1", [128, 3072], BF16).ap()
    qT2t = nc.alloc_sbuf_tensor("qT2t", [128, 1536], BF16).ap()
    v_all = nc.alloc_sbuf_tensor("v_all", [128, JC], BF16).ap()

    def qT_sl(i0, w=512):
        if i0 < 1024:
            assert i0 + w <= 1024
            return qk0[:, i0:i0 + w]
        if i0 < 2560:
            assert i0 + w <= 2560
            return qk1[:, i0 - 1024:i0 - 1024 + w]
        return qT2t[:, i0 - 2560:i0 - 2560 + w]

    def kT_sl(jb):
        if jb < 8:
            return qk0[:, 1024 + jb * 128:1024 + (jb + 1) * 128]
        return qk1[:, 1536 + (jb - 8) * 128:1536 + (jb - 7) * 128]

    def v_sl(jb):
        return v_all[:, jb * 128:(jb + 1) * 128]

    v8 = nc.alloc_sbuf_tensor("v8", [128, JC], FP8).ap()       # (j, e) fp8
    zacc = nc.alloc_sbuf_tensor("zacc", [128, 40], F32).ap()
    zsum = nc.alloc_sbuf_tensor("zsum", [128, 16], F32).ap()
    ztmp = nc.alloc_sbuf_tensor("ztmp", [128, 16], F32).ap()
    szinv = nc.alloc_sbuf_tensor("szinv", [128, 16], F32).ap()  # 4096/Z per jb
    cs_row = nc.alloc_sbuf_tensor("cs_row", [1, 128], BF16).ap()  # colsum(v) row
    cs_bc = nc.alloc_sbuf_tensor("cs_bc", [128, 512], BF16).ap()  # bcast x4
    bo_row = nc.alloc_sbuf_tensor("bo_row", [1, C], BF16).ap()    # b_out row
    ones_r = nc.alloc_sbuf_tensor("ones_r", [1, 512], BF16).ap()
    out2g = [nc.alloc_sbuf_tensor(f"out2g{g}", [128, 512], BF16).ap()
             for g in range(8)]
    # dPn8: per jb, column i' = hb*128 + e2 holds data for row i = e2*32 + hb.
    P8 = nc.alloc_sbuf_tensor("P8", [128, NJB * HW], FP8).ap()
    # bf16 exp staging: 8 dedicated lead buffers + 6-deep ring for the rest.
    Plead = nc.alloc_sbuf_tensor("Plead", [128, 8 * 1024], BF16).ap()
    PR = 8
    Pring = nc.alloc_sbuf_tensor("Pring", [128, PR * 2048], BF16).ap()
    # f32 staging for DVE fast-exp (Schraudolph) chunks: exp(x) ~=
    # bitcast(int32(A*x + B)); written int32, read back as f32.
    PS32 = nc.alloc_sbuf_tensor("PS32", [128, 2 * 2048], F32).ap()
    SCH_A = 12102203.1615 * SCALE    # 2^23/ln2, score scale folded in
    SCH_B = 1064866805.0             # 127*2^23 - 486411 (min-max-rel-err)

    # zacc slot map: jb<4 -> 3 chunks (A,B,C leads+main); jb4-15 -> 2 (H0,H1)
    def zslot(jb, i0):
        if jb < 4:
            return 3 * jb + (0 if i0 == 0 else (1 if i0 == 1024 else 2))
        return 12 + 2 * (jb - 4) + (0 if i0 == 0 else 1)

    def nchunks(jb):
        return 3 if jb < 4 else 2

    def p8_dst(jb, i0, w):
        # out AP dims (e2: w/32 @1 offset i0/32, hb: 32 @128) within jb region
        reg = P8[:, jb * HW:(jb + 1) * HW]
        v = reg.rearrange("p (hb e) -> p e hb", hb=32)
        return v[:, i0 // 32:(i0 + w) // 32, :]

    def p8_lhsT(t, hb):
        # stationary for (pair t, hb-block): dims (p, jb-pair 2 @4096, 128 @1)
        reg = P8[:, t * 2 * HW:(t + 1) * 2 * HW]
        v = reg.rearrange("p (two x) -> p two x", two=2)
        return v[:, :, hb * 128:(hb + 1) * 128]

    with TileContext(nc) as tc:
        with tc.tile_pool(name="consts", bufs=1) as consts:
            # ---- constants: ONE merged DMA for wq (gates stage-1, sync ring
            #      first so it lands before x), rest merged on scalar ----
            wq_all = consts.tile([128, 2 * O], BF16, name="wq_all", tag="wq_all")
            wqT = [wq_all[:, 0:O], wq_all[:, O:2 * O]]
            wq_ld = nc.sync.dma_start(
                out=wq_all[:].rearrange("p (cb o) -> p cb o", cb=2),
                in_=wqkvT_ext.rearrange("(cb p) o -> p cb o", cb=2))
            bias = consts.tile([128, 8], F32, name="bias", tag="bias")
            bq = [bias[:, i:i + 1] for i in range(3)]
            bo = [bias[:, 3 + i:4 + i] for i in range(2)]
            nc.scalar.dma_start(
                out=bias[:, 0:3],
                in_=bqkv_ext.rearrange("(a p) one -> p (a one)", p=128))
            nc.scalar.dma_start(
                out=bias[:, 3:5],
                in_=bout_ext.rearrange("(cb p) one -> p (cb one)", p=128))
            misc = consts.tile([128, C + 8], BF16, name="misc", tag="misc")
            woutT = misc[:, 0:C]
            ones16 = misc[:, C:C + 1]
            nc.scalar.dma_start(out=woutT, in_=woutT_ext[:])
            nc.vector.memset(ones16, 1.0)
            nc.vector.memset(ones_r[:], 1.0)
            nc.scalar.dma_start(out=bo_row[:], in_=boutr_ext[:])
            scratch = consts.tile([128, 2], F32, name="scratch", tag="scratch")
            neg1 = scratch[:, 1:2]
            nc.vector.memset(scratch[:, 0:1], 0.0)
            nc.vector.memset(neg1, -1.0)
            nc.scalar.activation(scratch[:, 0:1], scratch[:, 0:1], AF.Exp)

            # ---- PE warmup: dummy matmuls so HAM ramps early ----
            wsrc = consts.tile([128, 128], BF16, name="wsrc", tag="wsrc")
            nc.vector.memset(wsrc[:], 1.0)
            with tc.tile_pool(name="psW", bufs=1, space="PSUM") as psW:
                wtile = psW.tile([128, 128], F32, tag="warm")
                for _ in range(16):
                    nc.tensor.matmul(wtile[:], wsrc[:], wsrc[:], start=True, stop=True)

            # ---- x loads: all on sync ring (SP has nothing else to do) ----
            early = tc.alloc_tile_pool(name="early", bufs=1)
            xsb = [[early.tile([128, HW // 2], BF16, name=f"x{cb}{h}", tag=f"x{cb}{h}")
                    for h in range(2)] for cb in range(2)]
            Fsb = [early.tile([128, HW], BF16, name=f"F{i}", tag=f"F{i}")
                   for i in range(3)]
            # all 4 x tiles upfront (dep-free DMAs dispatch back-to-back;
            # putting F writes between them head-of-line blocks the ring)
            xlds = []
            for h in range(2):
                for cb in range(2):
                    r = nc.sync.dma_start(
                        out=xsb[cb][h][:],
                        in_=x_ext[cb * 128:(cb + 1) * 128,
                                  h * (HW // 2):(h + 1) * (HW // 2)])
                    add_dep_helper(r.ins, (xlds[-1] if xlds else wq_ld).ins,
                                   sync=False, reason="ring order")
                    xlds.append(r)

            # ---- stage 1: F o-tiles -> fbuf, transposes interleaved on the
            #      sync ring right after the F writes they depend on ----
            # drains: h0 chunks -> DVE; h1 chunks: ob0 -> ACT, ob1/ob2 -> Pool
            # (keeps ACT free for lead exps after ~7us).
            tr_specs = {
                "qk0": (qk0.rearrange("p (b i) -> p b i", b=2),
                        fview_i[0:1024, 0:2 * E], (0,)),
                "qk1": (qk1.rearrange("p (b i) -> p b i", b=2),
                        fview_i[1024:2560, 0:2 * E], (0, 1)),
                "qT2": (qT2t, fview_i[2560:HW, 0:E], (1, 2)),
            }
            # psL (lead-chunk PSUM) opened BEFORE psF so its banks are
            # disjoint from stage-1's: no anti-deps delaying the lead exps.
            psL = tc.alloc_tile_pool(name="psL", bufs=2, space="PSUM")
            with tc.tile_pool(name="psF", bufs=4, space="PSUM") as psF:
                f_writes = [[] for _ in range(3)]
                ring_last = [xlds[-1]]  # last sync-ring DMA, for order hints

                tr_done = {}

                def launch_tr(key):
                    dst, srcap, deps = tr_specs[key]
                    rt = nc.sync.dma_start_transpose(out=dst[:], in_=srcap)
                    for dep in deps:
                        for w2 in f_writes[dep]:
                            add_dep_helper(rt.ins, w2.ins, reason="fbuf RAW")
                    add_dep_helper(rt.ins, ring_last[0].ins, sync=False,
                                   reason="ring order")
                    ring_last[0] = rt
                    tr_done[key] = rt

                for ob in range(3):
                    for nch in range(8):
                        pf = psF.tile([128, 512], F32, tag="pf")
                        h, o512 = nch // 4, (nch % 4) * 512
                        nc.tensor.matmul(
                            pf[:], wqT[0][:, ob * 128:(ob + 1) * 128],
                            xsb[0][h][:, o512:o512 + 512],
                            start=True, stop=False,
                        )
                        nc.tensor.matmul(
                            pf[:], wqT[1][:, ob * 128:(ob + 1) * 128],
                            xsb[1][h][:, o512:o512 + 512],
                            start=False, stop=True,
                        )
                        fb = Fsb[ob]
                        fsl = slice(nch * 512, (nch + 1) * 512)
                        # GPSIMD cannot read PSUM on real HW: DVE/ACT only.
                        # 2+2 split per half-tile: both engines work in
                        # parallel so each F half is drained ~1.3us after
                        # its last MM instead of 2.4us serial.
                        if nch % 4 < 2:
                            nc.vector.tensor_scalar_add(fb[:, fsl], pf[:], bq[ob])
                        else:
                            nc.scalar.activation(fb[:, fsl], pf[:], AF.Identity,
                                                 bias=bq[ob])
                        if nch == 3 or nch == 7:
                            hh = nch // 4
                            fw = nc.sync.dma_start(
                                out=fview_o[ob * 128:(ob + 1) * 128,
                                            hh * (HW // 2):(hh + 1) * (HW // 2)],
                                in_=fb[:, hh * (HW // 2):(hh + 1) * (HW // 2)],
                            )
                            add_dep_helper(fw.ins, ring_last[0].ins, sync=False,
                                           reason="ring order")
                            ring_last[0] = fw
                            f_writes[ob].append(fw)
                    launch_tr(["qk0", "qk1", "qT2"][ob])
                # v loads (SWDGE) issued last: only needed in the tail, and
                # their gather transfers must not delay the transposes on the
                # serialized DMA device.
                r = nc.gpsimd.dma_start(
                    out=v_all.rearrange("p (t d) -> p t d", d=128),
                    in_=fview_i[0:JC, 2 * E:3 * E].rearrange(
                        "(t p) d -> p t d", p=128))
                for w2 in f_writes[0] + f_writes[1]:
                    add_dep_helper(r.ins, w2.ins, reason="fbuf RAW")
                # don't let the 14us v gather beat the transposes to the
                # serialized DMA device
                add_dep_helper(r.ins, tr_done["qT2"].ins, sync=False,
                               reason="DMA order: v after transposes")
            early.release()

            # v8 = fp8(v) plain convert (DVE), as soon as v lands
            nc.vector.tensor_copy(v8[:], v_all[:])

            # ---- phase A ----
            def score_exp(pool, jb, i0, w, buf, dve_fast=False):
                pa = pool.tile([128, w], F32, tag="pa")
                for n2 in range(w // 512):
                    nc.tensor.matmul(
                        pa[:, n2 * 512:(n2 + 1) * 512],
                        kT_sl(jb), qT_sl(i0 + n2 * 512),
                        start=True, stop=True,
                    )
                zs = zslot(jb, i0)
                if dve_fast:
                    # Schraudolph fast exp on DVE: offloads the ACT
                    # bottleneck; the Z row-sum runs on Pool (SBUF->SBUF)
                    # so neither helper engine eats the full cost.
                    nc.vector.tensor_scalar(
                        out=buf.bitcast(mybir.dt.int32), in0=pa[:],
                        scalar1=SCH_A, scalar2=SCH_B,
                        op0=ALU.mult, op1=ALU.add,
                    )
                    nc.vector.reduce_sum(
                        out=zacc[:, zs:zs + 1], in_=buf,
                        axis=mybir.AxisListType.X,
                    )
                else:
                    nc.scalar.activation(
                        out=buf, in_=pa[:], func=AF.Exp, scale=SCALE,
                        accum_out=zacc[:, zs:zs + 1],
                    )

            def stats(jb):
                k = nchunks(jb)
                s0 = zslot(jb, 0)
                nc.vector.reduce_sum(
                    out=zsum[:, jb:jb + 1], in_=zacc[:, s0:s0 + k],
                    axis=mybir.AxisListType.X,
                )
                nc.vector.tensor_scalar_mul(
                    ztmp[:, jb:jb + 1], zsum[:, jb:jb + 1], 1.0 / SFIX)
                nc.vector.reciprocal(szinv[:, jb:jb + 1], ztmp[:, jb:jb + 1])

            def convert(jb, i0, w, buf, eng):
                src = buf.rearrange("p (e hb) -> p e hb", hb=32)
                dst = p8_dst(jb, i0, w)
                if eng is nc.scalar:
                    nc.scalar.activation(dst, src, AF.Identity,
                                         scale=szinv[:, jb:jb + 1], bias=neg1)
                else:
                    eng.tensor_scalar(
                        out=dst, in0=src,
                        scalar1=szinv[:, jb:jb + 1], scalar2=1.0,
                        op0=ALU.mult, op1=ALU.subtract,
                    )

            def conv_eng(jb):
                # alternate DVE/Pool so neither falls behind the exp stream
                return nc.gpsimd if (jb >= 4 and jb % 2 == 0) else nc.vector

            # lead chunks: jb0-3, i<2048 as 1024-wide halves (only need
            # qT part0/1 + kT part0); dedicated buffers (no ring WAR).
            for li, (jb, i0) in enumerate(
                    [(j, 0) for j in range(4)] + [(j, 1024) for j in range(4)]):
                buf = Plead[:, li * 1024:(li + 1) * 1024]
                score_exp(psL, jb, i0, 1024, buf)
            psL.release()

            # main chunks: jb0-3 C first, then jb4-15 pairs (2048 wide)
            mains = [(jb, 2048, 2048) for jb in range(4)]
            for jb in range(4, 16):
                mains += [(jb, 0, 2048), (jb, 2048, 2048)]
            seq = 0
            prev_buf = [None]

            def ring_buf():
                b = Pring[:, (seq % PR) * 2048:(seq % PR + 1) * 2048]
                return b

            SCHRAUD = set()
            sch_i = [0]
            pending = []   # deferred convert emissions (deadline: the tail)

            def flush_pending():
                for args in pending:
                    convert(*args)
                pending.clear()

            with tc.tile_pool(name="psA", bufs=2, space="PSUM") as psA:
                for (jb, i0, w) in mains:
                    fast = (jb, i0) in SCHRAUD
                    if fast:
                        s = (sch_i[0] % 2) * 2048
                        buf = PS32[:, s:s + 2048]
                        sch_i[0] += 1
                    else:
                        buf = ring_buf()
                        seq += 1
                    score_exp(psA, jb, i0, w, buf[:, 0:w], dve_fast=fast)
                    if i0 == 0:
                        # DVE ops for the NEXT chunk must enqueue before the
                        # previous jb's DVE converts (strict FIFO) or the
                        # psA slot recycle stalls PE/ACT
                        flush_pending()
                    if i0 + w == HW:     # jb complete -> stats + converts
                        stats(jb)
                        if jb < 4:
                            lA = Plead[:, (jb) * 1024:(jb + 1) * 1024]
                            lB = Plead[:, (4 + jb) * 1024:(5 + jb) * 1024]
                            pending.append((jb, 0, 1024, lA, nc.vector))
                            pending.append((jb, 1024, 1024, lB, nc.vector))
                            pending.append((jb, 2048, 2048, buf, nc.vector))
                        else:
                            e0 = nc.scalar if jb == 15 else conv_eng(jb)
                            pending.append((jb, 0, 2048, prev_buf[0], e0))
                            pending.append(
                                (jb, 2048, 2048, buf,
                                 nc.vector if jb == 15 else conv_eng(jb)))
                            if jb == 15:
                                flush_pending()
                    prev_buf[0] = buf
                flush_pending()

            # ---- late phase A + tail.  out2 is produced DIRECTLY by fp8
            #      DoubleRow pair-MMs (P8 stationary): out2[e2, hb*128+e] =
            #      sum_j Pn[j,..]*v8[j,e].  Groups 0-3 accumulate pairs 0-6
            #      (jb<14) in psE WHILE jb14-15 exps run in the 4 freed
            #      banks (1024-wide chunks); the rank-1 colsum seed + pair 7
            #      join in the tail (PSUM accumulation order is free). ----
            ones_row = wsrc[0:1, :]

            def v8pair_ap(t):
                return v8[:, t * 256:(t + 1) * 256].rearrange(
                    "p (two e) -> p two e", two=2)

            with tc.tile_pool(name="psCS", bufs=1, space="PSUM") as psCS, \
                 tc.tile_pool(name="psB", bufs=3, space="PSUM") as psB, \
                 tc.tile_pool(name="psY", bufs=2, space="PSUM") as psY, \
                 tc.tile_pool(name="late", bufs=1) as late:
                # colsum row: cs[0, e] = sum_j v[j, e] in bf16
                cs_ps = psCS.tile([1, 128], F32, tag="cs")
                for jb in range(NJB):
                    nc.tensor.matmul(
                        cs_ps[:], ones16, v_sl(jb),
                        start=(jb == 0), stop=(jb == NJB - 1),
                    )
                nc.vector.tensor_copy(cs_row[:], cs_ps[:])
                # broadcast the colsum row to all partitions (Pool, SBUF only)
                # so the rank-1 "1"-term joins at DRAIN time instead of as 32
                # K=1 seed matmuls on the PE
                for s in range(4):
                    nc.gpsimd.partition_broadcast(
                        cs_bc[:, s * 128:(s + 1) * 128], cs_row[:])

                yg = [late.tile([128, 1024], BF16, name=f"yg{g}", tag=f"yg{g}")
                      for g in range(8)]
                for g in range(8):
                    ob_ps = psB.tile([128, 512], F32, tag="ob_ps")
                    for s in range(4):
                        sl = ob_ps[:, s * 128:(s + 1) * 128]
                        for t in range(NJB // 2):
                            nc.tensor.matmul(
                                sl, p8_lhsT(t, 4 * g + s), v8pair_ap(t),
                                start=(t == 0), stop=(t == NJB // 2 - 1),
                                perf_mode=DR,
                            )
                    nc.vector.tensor_add(out2g[g][:], ob_ps[:], cs_bc[:])
                    # proj2 for BOTH c-halves into one (128,1024) PSUM tile;
                    # b_out joins as a K=1 rank-1 matmul, so the drain is a
                    # single plain ACT copy and y moves in ONE DMA per group.
                    py = psY.tile([128, 1024], F32, tag="py")
                    for cb in range(2):
                        psl = py[:, cb * 512:(cb + 1) * 512]
                        nc.tensor.matmul(
                            psl, woutT[:, cb * 128:(cb + 1) * 128], out2g[g][:],
                            start=True, stop=False,
                        )
                        nc.tensor.matmul(
                            psl, bo_row[0:1, cb * 128:(cb + 1) * 128],
                            ones_r[:], start=False, stop=True,
                        )
                    ygt = yg[g][:]
                    if g % 2 == 0:
                        nc.scalar.activation(ygt, py[:], AF.Identity)
                    else:
                        nc.vector.tensor_copy(ygt, py[:])
                    nc.sync.dma_start(
                        out=y_ext.rearrange("(cb p) hw -> p cb hw", cb=2)[
                            :, :, g * 512:(g + 1) * 512],
                        in_=ygt.rearrange("p (cb x) -> p cb x", cb=2))

    nc.compile()
    return nc


def get_nc():
    if "nc" not in _CACHE:
        _CACHE["nc"] = build_nc()
    return _CACHE["nc"]


def make_in_maps(x, W_qkv, b_qkv, W_out, b_out):
    x = np.asarray(x, dtype=np.float32)
    W_qkv = np.asarray(W_qkv, dtype=np.float32)
    b_qkv = np.asarray(b_qkv, dtype=np.float32)
    W_out = np.asarray(W_out, dtype=np.float32) / SFIX   # undo Pn renorm
    b_out = np.asarray(b_out, dtype=np.float32)

    operm = (np.arange(O) + O // 2) % O      # rotate qkv channels by 192
    eperm = (np.arange(E) + E // 2) % E      # rotate e-axis by 64

    halves = []
    for h in range(2):
        if h == 0:
            wq, bqv, wo, bov = W_qkv, b_qkv, W_out, b_out
        else:
            wq = W_qkv[operm]
            bqv = b_qkv[operm]
            wo = W_out[:, eperm]
            bov = np.zeros_like(b_out)
        halves.append({
            "wqkvT": np.ascontiguousarray(wq.T).astype(ml_dtypes.bfloat16),
            "bqkv": np.ascontiguousarray(bqv.reshape(O, 1)),
            "woutT": np.ascontiguousarray(wo.T).astype(ml_dtypes.bfloat16),
            "bout": np.ascontiguousarray(bov.reshape(C, 1)),
            "boutr": np.ascontiguousarray(bov.reshape(1, C)).astype(
                ml_dtypes.bfloat16),
        })

    xb = [np.ascontiguousarray(x[n].reshape(C, HW)).astype(ml_dtypes.bfloat16)
          for n in range(N)]
    in_maps = []
    for core in range(8):
        n, h = core // 2, core % 2
        m = {"x": xb[n]}
        m.update(halves[h])
        in_maps.append(m)
    return in_maps


def run(inputs, trace=False, **kw):
    nc = get_nc()
    in_maps = make_in_maps(**inputs)
    res = run_bass_kernel_spmd(nc, in_maps, core_ids=list(range(8)), trace=trace, **kw)
    ys = [np.asarray(res.results[i]["out"], dtype=np.float32) for i in range(8)]
    y = np.stack([ys[2 * n] + ys[2 * n + 1] for n in range(N)])
    return y.reshape(N, C, 64, 64), res


def kernel(**inputs):
    y, _ = run(inputs, trace=False)
    return y


# revision 91
# speedup vs baseline: 1.0321x; 1.0321x over previous
"""Trainium2 Bass kernel for nn_Attention (dense_transformer).

Reference computation (per batch n of 4):
  qkv = W_qkv @ x + b          (384, 4096)   [x flattened to (256, 64*64)]
  raw C-order reinterpret of qkv flat buffer as (4096, 384) -> q|k|v (4096,128) each
  scores = q @ k.T / 64        (4096, 4096)
  soft = softmax(scores, axis=-2)             [column softmax]
  out = soft @ v               (4096, 128)
  raw reinterpret of out as (128, 4096)
  y = W_out @ out2 + b_out     (256, 4096)

Sharding: 8 cores = 4 batches x 2 column-chunks (j-axis of the score
matrix = rows of k/v).  Column-softmax stats are local to a j-chunk;
each core produces a partial y, host sums the pair.  The SPMD graph is
identical on all cores; the j-half selection is encoded host-side by
rotating the qkv output channels by 192 for odd cores and rotating
W_out's e-axis by 64 to compensate.

Compute layout (per core), final:
  head:    merged-const + x loads and F = W_qkv@x+b (bf16) -> fbuf
           half-tile writes interleaved with MERGED q|k xbar
           transposes (qk0/qk1/qT2) on the sync ring, with explicit
           sync=False ring-order hints (the DMA device is an exclusive
           serial resource; per-DMA dispatch latency ~2us).  F drains
           split 2 DVE + 2 ACT per half-tile.  v gather via SWDGE,
           ordered after the transposes (tail-only data).
  phase A: ACT runs ONLY exps (the critical path, ~68us).  Lead
           chunks (jb0-3 x 1024-wide halves of i<2048, qk0/qk1 only)
           start ~20us; then 28 x 2048-wide chunks (psA = 2 x 4 PSUM
           banks).  Each exp: 4 score MMs bf16 -> PSUM, one exp with
           accum_out -> bf16 ring (PR=8 slots, WAR-safe via deferred
           converts).
  stats:   DVE: zsum = reduce(zacc); szinv = 1/(zsum/4096) = 4096/Z.
  converts (DVE/Pool alternating, ACT for the tail-gating jb15-H0,
           emission deferred one chunk so DVE's FIFO never blocks the
           psA recycle): dPn8 = fp8e4(P*szinv - 1) scattered into P8
           so column i' = hb*128+e2 holds score row i = e2*32+hb.
           v8 = fp8e4(v) plain.
  tail:    colsum row cs[e] = sum_j v_bf16[j,e] (16 rank-reducing MMs
           to a (1,128) PSUM row; the rank-1 "1" term of Pn = 1+dPn);
           per 512-wide group g, per hb: K=1 seed MM (ones x cs_row)
           + 8 fp8 DoubleRow pair-MMs with P8 STATIONARY, which
           produce out2[e2, hb*128+e] DIRECTLY in PSUM (no TensorE
           transposes needed); drain ACT/DVE, proj2 (woutT pre-scaled
           by 1/4096 host-side) + b_out, y -> sync + gpsimd rings.
"""

import numpy as np
import ml_dtypes

import concourse.bass as bass
import concourse.bacc as bacc
import concourse.mybir as mybir
from concourse.bass_utils import run_bass_kernel_spmd
from concourse.tile import TileContext, add_dep_helper
from concourse.masks import make_identity

BF16 = mybir.dt.bfloat16
F32 = mybir.dt.float32
FP8 = mybir.dt.float8e4
AF = mybir.ActivationFunctionType
ALU = mybir.AluOpType
DR = mybir.MatmulPerfMode.DoubleRow

N, C, E, O, HW = 4, 256, 128, 384, 4096
JC = HW // 2          # j-chunk per core
NJB = JC // 128       # 16 j-blocks
SCALE = 1.0 / 64.0    # 1/sqrt(HW)
SFIX = 4096.0         # softmax renorm: Pn = P * (4096/Z), undone in W_out

_CACHE = {}


def build_nc():
    nc = bacc.Bacc("TRN2", target_bir_lowering=False, debug=False, num_devices=8)

    x_ext = nc.dram_tensor("x", [C, HW], BF16, kind="ExternalInput").ap()
    wqkvT_ext = nc.dram_tensor("wqkvT", [C, O], BF16, kind="ExternalInput").ap()
    bqkv_ext = nc.dram_tensor("bqkv", [O, 1], F32, kind="ExternalInput").ap()
    woutT_ext = nc.dram_tensor("woutT", [E, C], BF16, kind="ExternalInput").ap()
    bout_ext = nc.dram_tensor("bout", [C, 1], F32, kind="ExternalInput").ap()
    y_ext = nc.dram_tensor("out", [C, HW], BF16, kind="ExternalOutput").ap()

    fbuf = nc.dram_tensor("fbuf", [O * HW], BF16).ap()
    fview_o = fbuf.rearrange("(o hw) -> o hw", hw=HW)   # (384, 4096) write view
    fview_i = fbuf.rearrange("(i j) -> i j", j=O)        # (4096, 384) read view

    # persistent SBUF.  qT/kT/v split at 512-aligned boundaries covered by
    # successive F o-tiles.
    # merged q|k transpose targets: qk0 = rows 0-1024 x cols 0-256 of the
    # (i, j) view (q part 0 | k part 0), qk1 = rows 1024-2560 (q part 1 |
    # k part 1 + 512 junk), qT2t = rows 2560-4096 x q only.
    qk0 = nc.alloc_sbuf_tensor("qk0", [128, 2048], BF16).ap()
    qk1 = nc.alloc_sbuf_tensor("qk1", [128, 3072], BF16).ap()
    qT2t = nc.alloc_sbuf_tensor("qT2t", [128, 1536], BF16).ap()
    v_all = nc.alloc_sbuf_tensor("v_all", [128, JC], BF16).ap()

    def qT_sl(i0, w=512):
        if i0 < 1024:
            assert i0 + w <= 1024
            return qk0[:, i0:i0 + w]
        if i0 < 2560:
            assert i0 + w <= 2560
            return qk1[:, i0 - 1024:i0 - 1024 + w]
        return qT2t[:, i0 - 2560:i0 - 2560 + w]

    def kT_sl(jb):
        if jb < 8:
            return qk0[:, 1024 + jb * 128:1024 + (jb + 1) * 128]
        return qk1[:, 1536 + (jb - 8) * 128:1536 + (jb - 7) * 128]

    def v_sl(jb):
        return v_all[:, jb * 128:(jb + 1) * 128]

    v8 = nc.alloc_sbuf_tensor("v8", [128, JC], FP8).ap()       # (j, e) fp8
    zacc = nc.alloc_sbuf_tensor("zacc", [128, 40], F32).ap()
    zsum = nc.alloc_sbuf_tensor("zsum", [128, 16], F32).ap()
    ztmp = nc.alloc_sbuf_tensor("ztmp", [128, 16], F32).ap()
    szinv = nc.alloc_sbuf_tensor("szinv", [128, 16], F32).ap()  # 4096/Z per jb
    cs_row = nc.alloc_sbuf_tensor("cs_row", [1, 128], BF16).ap()  # colsum(v) row
    cs_bc = nc.alloc_sbuf_tensor("cs_bc", [128, 512], BF16).ap()  # bcast x4
    out2g = [nc.alloc_sbuf_tensor(f"out2g{g}", [128, 512], BF16).ap()
             for g in range(8)]
    # dPn8: per jb, column i' = hb*128 + e2 holds data for row i = e2*32 + hb.
    P8 = nc.alloc_sbuf_tensor("P8", [128, NJB * HW], FP8).ap()
    # bf16 exp staging: 8 dedicated lead buffers + 6-deep ring for the rest.
    Plead = nc.alloc_sbuf_tensor("Plead", [128, 8 * 1024], BF16).ap()
    PR = 8
    Pring = nc.alloc_sbuf_tensor("Pring", [128, PR * 2048], BF16).ap()
    # f32 staging for DVE fast-exp (Schraudolph) chunks: exp(x) ~=
    # bitcast(int32(A*x + B)); written int32, read back as f32.
    PS32 = nc.alloc_sbuf_tensor("PS32", [128, 2 * 2048], F32).ap()
    SCH_A = 12102203.1615 * SCALE    # 2^23/ln2, score scale folded in
    SCH_B = 1064866805.0             # 127*2^23 - 486411 (min-max-rel-err)

    # zacc slot map: jb<4 -> 3 chunks (A,B,C leads+main); jb4-15 -> 2 (H0,H1)
    def zslot(jb, i0):
        if jb < 4:
            return 3 * jb + (0 if i0 == 0 else (1 if i0 == 1024 else 2))
        return 12 + 2 * (jb - 4) + (0 if i0 == 0 else 1)

    def nchunks(jb):
        return 3 if jb < 4 else 2

    def p8_dst(jb, i0, w):
        # out AP dims (e2: w/32 @1 offset i0/32, hb: 32 @128) within jb region
        reg = P8[:, jb * HW:(jb + 1) * HW]
        v = reg.rearrange("p (hb e) -> p e hb", hb=32)
        return v[:, i0 // 32:(i0 + w) // 32, :]

    def p8_lhsT(t, hb):
        # stationary for (pair t, hb-block): dims (p, jb-pair 2 @4096, 128 @1)
        reg = P8[:, t * 2 * HW:(t + 1) * 2 * HW]
        v = reg.rearrange("p (two x) -> p two x", two=2)
        return v[:, :, hb * 128:(hb + 1) * 128]

    with TileContext(nc) as tc:
        with tc.tile_pool(name="consts", bufs=1) as consts:
            # ---- constants: ONE merged DMA for wq (gates stage-1, sync ring
            #      first so it lands before x), rest merged on scalar ----
            wq_all = consts.tile([128, 2 * O], BF16, name="wq_all", tag="wq_all")
            wqT = [wq_all[:, 0:O], wq_all[:, O:2 * O]]
            wq_ld = nc.sync.dma_start(
                out=wq_all[:].rearrange("p (cb o) -> p cb o", cb=2),
                in_=wqkvT_ext.rearrange("(cb p) o -> p cb o", cb=2))
            bias = consts.tile([128, 8], F32, name="bias", tag="bias")
            bq = [bias[:, i:i + 1] for i in range(3)]
            bo = [bias[:, 3 + i:4 + i] for i in range(2)]
            nc.scalar.dma_start(
                out=bias[:, 0:3],
                in_=bqkv_ext.rearrange("(a p) one -> p (a one)", p=128))
            nc.scalar.dma_start(
                out=bias[:, 3:5],
                in_=bout_ext.rearrange("(cb p) one -> p (cb one)", p=128))
            misc = consts.tile([128, C + 8], BF16, name="misc", tag="misc")
            woutT = misc[:, 0:C]
            ones16 = misc[:, C:C + 1]
            nc.scalar.dma_start(out=woutT, in_=woutT_ext[:])
            nc.vector.memset(ones16, 1.0)
            scratch = consts.tile([128, 2], F32, name="scratch", tag="scratch")
            neg1 = scratch[:, 1:2]
            nc.vector.memset(scratch[:, 0:1], 0.0)
            nc.vector.memset(neg1, -1.0)
            nc.scalar.activation(scratch[:, 0:1], scratch[:, 0:1], AF.Exp)

            # ---- PE warmup: dummy matmuls so HAM ramps early ----
            wsrc = consts.tile([128, 128], BF16, name="wsrc", tag="wsrc")
            nc.vector.memset(wsrc[:], 1.0)
            with tc.tile_pool(name="psW", bufs=1, space="PSUM") as psW:
                wtile = psW.tile([128, 128], F32, tag="warm")
                for _ in range(16):
                    nc.tensor.matmul(wtile[:], wsrc[:], wsrc[:], start=True, stop=True)

            # ---- x loads: all on sync ring (SP has nothing else to do) ----
            early = tc.alloc_tile_pool(name="early", bufs=1)
            xsb = [[early.tile([128, HW // 2], BF16, name=f"x{cb}{h}", tag=f"x{cb}{h}")
                    for h in range(2)] for cb in range(2)]
            Fsb = [early.tile([128, HW], BF16, name=f"F{i}", tag=f"F{i}")
                   for i in range(3)]
            # all 4 x tiles upfront (dep-free DMAs dispatch back-to-back;
            # putting F writes between them head-of-line blocks the ring)
            xlds = []
            for h in range(2):
                for cb in range(2):
                    r = nc.sync.dma_start(
                        out=xsb[cb][h][:],
                        in_=x_ext[cb * 128:(cb + 1) * 128,
                                  h * (HW // 2):(h + 1) * (HW // 2)])
                    add_dep_helper(r.ins, (xlds[-1] if xlds else wq_ld).ins,
                                   sync=False, reason="ring order")
                    xlds.append(r)

            # ---- stage 1: F o-tiles -> fbuf, transposes interleaved on the
            #      sync ring right after the F writes they depend on ----
            # drains: h0 chunks -> DVE; h1 chunks: ob0 -> ACT, ob1/ob2 -> Pool
            # (keeps ACT free for lead exps after ~7us).
            tr_specs = {
                "qk0": (qk0.rearrange("p (b i) -> p b i", b=2),
                        fview_i[0:1024, 0:2 * E], (0,)),
                "qk1": (qk1.rearrange("p (b i) -> p b i", b=2),
                        fview_i[1024:2560, 0:2 * E], (0, 1)),
                "qT2": (qT2t, fview_i[2560:HW, 0:E], (1, 2)),
            }
            # psL (lead-chunk PSUM) opened BEFORE psF so its banks are
            # disjoint from stage-1's: no anti-deps delaying the lead exps.
            psL = tc.alloc_tile_pool(name="psL", bufs=2, space="PSUM")
            with tc.tile_pool(name="psF", bufs=4, space="PSUM") as psF:
                f_writes = [[] for _ in range(3)]
                ring_last = [xlds[-1]]  # last sync-ring DMA, for order hints

                tr_done = {}

                def launch_tr(key):
                    dst, srcap, deps = tr_specs[key]
                    rt = nc.sync.dma_start_transpose(out=dst[:], in_=srcap)
                    for dep in deps:
                        for w2 in f_writes[dep]:
                            add_dep_helper(rt.ins, w2.ins, reason="fbuf RAW")
                    add_dep_helper(rt.ins, ring_last[0].ins, sync=False,
                                   reason="ring order")
                    ring_last[0] = rt
                    tr_done[key] = rt

                for ob in range(3):
                    for nch in range(8):
                        pf = psF.tile([128, 512], F32, tag="pf")
                        h, o512 = nch // 4, (nch % 4) * 512
                        nc.tensor.matmul(
                            pf[:], wqT[0][:, ob * 128:(ob + 1) * 128],
                            xsb[0][h][:, o512:o512 + 512],
                            start=True, stop=False,
                        )
                        nc.tensor.matmul(
                            pf[:], wqT[1][:, ob * 128:(ob + 1) * 128],
                            xsb[1][h][:, o512:o512 + 512],
                            start=False, stop=True,
                        )
                        fb = Fsb[ob]
                        fsl = slice(nch * 512, (nch + 1) * 512)
                        # GPSIMD cannot read PSUM on real HW: DVE/ACT only.
                        # 2+2 split per half-tile: both engines work in
                        # parallel so each F half is drained ~1.3us after
                        # its last MM instead of 2.4us serial.
                        if nch % 4 < 2:
                            nc.vector.tensor_scalar_add(fb[:, fsl], pf[:], bq[ob])
                        else:
                            nc.scalar.activation(fb[:, fsl], pf[:], AF.Identity,
                                                 bias=bq[ob])
                        if nch == 3 or nch == 7:
                            hh = nch // 4
                            fw = nc.sync.dma_start(
                                out=fview_o[ob * 128:(ob + 1) * 128,
                                            hh * (HW // 2):(hh + 1) * (HW // 2)],
                                in_=fb[:, hh * (HW // 2):(hh + 1) * (HW // 2)],
                            )
                            add_dep_helper(fw.ins, ring_last[0].ins, sync=False,
                                           reason="ring order")
                            ring_last[0] = fw
                            f_writes[ob].append(fw)
                    launch_tr(["qk0", "qk1", "qT2"][ob])
                # v loads (SWDGE) issued last: only needed in the tail, and
                # their gather transfers must not delay the transposes on the
                # serialized DMA device.
                r = nc.gpsimd.dma_start(
                    out=v_all.rearrange("p (t d) -> p t d", d=128),
                    in_=fview_i[0:JC, 2 * E:3 * E].rearrange(
                        "(t p) d -> p t d", p=128))
                for w2 in f_writes[0] + f_writes[1]:
                    add_dep_helper(r.ins, w2.ins, reason="fbuf RAW")
                # don't let the 14us v gather beat the transposes to the
                # serialized DMA device
                add_dep_helper(r.ins, tr_done["qT2"].ins, sync=False,
                               reason="DMA order: v after transposes")
            early.release()

            # v8 = fp8(v) plain convert (DVE), as soon as v lands
            nc.vector.tensor_copy(v8[:], v_all[:])

            # ---- phase A ----
            def score_exp(pool, jb, i0, w, buf, dve_fast=False):
                pa = pool.tile([128, w], F32, tag="pa")
                for n2 in range(w // 512):
                    nc.tensor.matmul(
                        pa[:, n2 * 512:(n2 + 1) * 512],
                        kT_sl(jb), qT_sl(i0 + n2 * 512),
                        start=True, stop=True,
                    )
                zs = zslot(jb, i0)
                if dve_fast:
                    # Schraudolph fast exp on DVE: offloads the ACT
                    # bottleneck; the Z row-sum runs on Pool (SBUF->SBUF)
                    # so neither helper engine eats the full cost.
                    nc.vector.tensor_scalar(
                        out=buf.bitcast(mybir.dt.int32), in0=pa[:],
                        scalar1=SCH_A, scalar2=SCH_B,
                        op0=ALU.mult, op1=ALU.add,
                    )
                    nc.vector.reduce_sum(
                        out=zacc[:, zs:zs + 1], in_=buf,
                        axis=mybir.AxisListType.X,
                    )
                else:
                    nc.scalar.activation(
                        out=buf, in_=pa[:], func=AF.Exp, scale=SCALE,
                        accum_out=zacc[:, zs:zs + 1],
                    )

            def stats(jb):
                k = nchunks(jb)
                s0 = zslot(jb, 0)
                nc.vector.reduce_sum(
                    out=zsum[:, jb:jb + 1], in_=zacc[:, s0:s0 + k],
                    axis=mybir.AxisListType.X,
                )
                nc.vector.tensor_scalar_mul(
                    ztmp[:, jb:jb + 1], zsum[:, jb:jb + 1], 1.0 / SFIX)
                nc.vector.reciprocal(szinv[:, jb:jb + 1], ztmp[:, jb:jb + 1])

            def convert(jb, i0, w, buf, eng):
                src = buf.rearrange("p (e hb) -> p e hb", hb=32)
                dst = p8_dst(jb, i0, w)
                if eng is nc.scalar:
                    nc.scalar.activation(dst, src, AF.Identity,
                                         scale=szinv[:, jb:jb + 1], bias=neg1)
                else:
                    eng.tensor_scalar(
                        out=dst, in0=src,
                        scalar1=szinv[:, jb:jb + 1], scalar2=1.0,
                        op0=ALU.mult, op1=ALU.subtract,
                    )

            def conv_eng(jb):
                # alternate DVE/Pool so neither falls behind the exp stream
                return nc.gpsimd if (jb >= 4 and jb % 2 == 0) else nc.vector

            # lead chunks: jb0-3, i<2048 as 1024-wide halves (only need
            # qT part0/1 + kT part0); dedicated buffers (no ring WAR).
            for li, (jb, i0) in enumerate(
                    [(j, 0) for j in range(4)] + [(j, 1024) for j in range(4)]):
                buf = Plead[:, li * 1024:(li + 1) * 1024]
                score_exp(psL, jb, i0, 1024, buf)
            psL.release()

            # main chunks: jb0-3 C first, then jb4-15 pairs (2048 wide)
            mains = [(jb, 2048, 2048) for jb in range(4)]
            for jb in range(4, 16):
                mains += [(jb, 0, 2048), (jb, 2048, 2048)]
            seq = 0
            prev_buf = [None]

            def ring_buf():
                b = Pring[:, (seq % PR) * 2048:(seq % PR + 1) * 2048]
                return b

            SCHRAUD = set()
            sch_i = [0]
            pending = []   # deferred convert emissions (deadline: the tail)

            def flush_pending():
                for args in pending:
                    convert(*args)
                pending.clear()

            with tc.tile_pool(name="psA", bufs=2, space="PSUM") as psA:
                for (jb, i0, w) in mains:
                    fast = (jb, i0) in SCHRAUD
                    if fast:
                        s = (sch_i[0] % 2) * 2048
                        buf = PS32[:, s:s + 2048]
                        sch_i[0] += 1
                    else:
                        buf = ring_buf()
                        seq += 1
                    score_exp(psA, jb, i0, w, buf[:, 0:w], dve_fast=fast)
                    if i0 == 0:
                        # DVE ops for the NEXT chunk must enqueue before the
                        # previous jb's DVE converts (strict FIFO) or the
                        # psA slot recycle stalls PE/ACT
                        flush_pending()
                    if i0 + w == HW:     # jb complete -> stats + converts
                        stats(jb)
                        if jb < 4:
                            lA = Plead[:, (jb) * 1024:(jb + 1) * 1024]
                            lB = Plead[:, (4 + jb) * 1024:(5 + jb) * 1024]
                            pending.append((jb, 0, 1024, lA, nc.vector))
                            pending.append((jb, 1024, 1024, lB, nc.vector))
                            pending.append((jb, 2048, 2048, buf, nc.vector))
                        else:
                            e0 = nc.scalar if jb == 15 else conv_eng(jb)
                            pending.append((jb, 0, 2048, prev_buf[0], e0))
                            pending.append(
                                (jb, 2048, 2048, buf,
                                 nc.vector if jb == 15 else conv_eng(jb)))
                            if jb == 15:
                                flush_pending()
                    prev_buf[0] = buf
                flush_pending()

            # ---- late phase A + tail.  out2 is produced DIRECTLY by fp8
            #      DoubleRow pair-MMs (P8 stationary): out2[e2, hb*128+e] =
            #      sum_j Pn[j,..]*v8[j,e].  Groups 0-3 accumulate pairs 0-6
            #      (jb<14) in psE WHILE jb14-15 exps run in the 4 freed
            #      banks (1024-wide chunks); the rank-1 colsum seed + pair 7
            #      join in the tail (PSUM accumulation order is free). ----
            ones_row = wsrc[0:1, :]

            def v8pair_ap(t):
                return v8[:, t * 256:(t + 1) * 256].rearrange(
                    "p (two e) -> p two e", two=2)

            with tc.tile_pool(name="psCS", bufs=1, space="PSUM") as psCS, \
                 tc.tile_pool(name="psB", bufs=3, space="PSUM") as psB, \
                 tc.tile_pool(name="psY", bufs=3, space="PSUM") as psY, \
                 tc.tile_pool(name="late", bufs=1) as late:
                # colsum row: cs[0, e] = sum_j v[j, e] in bf16
                cs_ps = psCS.tile([1, 128], F32, tag="cs")
                for jb in range(NJB):
                    nc.tensor.matmul(
                        cs_ps[:], ones16, v_sl(jb),
                        start=(jb == 0), stop=(jb == NJB - 1),
                    )
                nc.vector.tensor_copy(cs_row[:], cs_ps[:])
                # broadcast the colsum row to all partitions (Pool, SBUF only)
                # so the rank-1 "1"-term joins at DRAIN time instead of as 32
                # K=1 seed matmuls on the PE
                for s in range(4):
                    nc.gpsimd.partition_broadcast(
                        cs_bc[:, s * 128:(s + 1) * 128], cs_row[:])

                yg = [[late.tile([128, 512], BF16, name=f"yb{cb}_{g}",
                                 tag=f"yb{cb}_{g}") for g in range(8)]
                      for cb in range(2)]
                for g in range(8):
                    ob_ps = psB.tile([128, 512], F32, tag="ob_ps")
                    for s in range(4):
                        sl = ob_ps[:, s * 128:(s + 1) * 128]
                        for t in range(NJB // 2):
                            nc.tensor.matmul(
                                sl, p8_lhsT(t, 4 * g + s), v8pair_ap(t),
                                start=(t == 0), stop=(t == NJB // 2 - 1),
                                perf_mode=DR,
                            )
                    nc.vector.tensor_add(out2g[g][:], ob_ps[:], cs_bc[:])
                    for cb in range(2):
                        py = psY.tile([128, 512], F32, tag="py")
                        nc.tensor.matmul(
                            py[:], woutT[:, cb * 128:(cb + 1) * 128], out2g[g][:],
                            start=True, stop=True,
                        )
                        dst = yg[cb][g][:]
                        if cb == 0:
                            nc.scalar.activation(dst, py[:], AF.Identity, bias=bo[cb])
                        else:
                            nc.vector.tensor_scalar_add(dst, py[:], bo[cb])
                        [nc.sync, nc.gpsimd][cb].dma_start(
                            out=y_ext[cb * 128:(cb + 1) * 128,
                                      g * 512:(g + 1) * 512],
                            in_=dst)

    nc.compile()
    return nc


def get_nc():
    if "nc" not in _CACHE:
        _CACHE["nc"] = build_nc()
    return _CACHE["nc"]


def make_in_maps(x, W_qkv, b_qkv, W_out, b_out):
    x = np.asarray(x, dtype=np.float32)
    W_qkv = np.asarray(W_qkv, dtype=np.float32)
    b_qkv = np.asarray(b_qkv, dtype=np.float32)
    W_out = np.asarray(W_out, dtype=np.float32) / SFIX   # undo Pn renorm
    b_out = np.asarray(b_out, dtype=np.float32)

    operm = (np.arange(O) + O // 2) % O      # rotate qkv channels by 192
    eperm = (np.arange(E) + E // 2) % E      # rotate e-axis by 64

    halves = []
    for h in range(2):
        if h == 0:
            wq, bqv, wo, bov = W_qkv, b_qkv, W_out, b_out
        else:
            wq = W_qkv[operm]
            bqv = b_qkv[operm]
            wo = W_out[:, eperm]
            bov = np.zeros_like(b_out)
        halves.append({
            "wqkvT": np.ascontiguousarray(wq.T).astype(ml_dtypes.bfloat16),
            "bqkv": np.ascontiguousarray(bqv.reshape(O, 1)),
            "woutT": np.ascontiguousarray(wo.T).astype(ml_dtypes.bfloat16),
            "bout": np.ascontiguousarray(bov.reshape(C, 1)),
        })

    xb = [np.ascontiguousarray(x[n].reshape(C, HW)).astype(ml_dtypes.bfloat16)
          for n in range(N)]
    in_maps = []
    for core in range(8):
        n, h = core // 2, core % 2
        m = {"x": xb[n]}
        m.update(halves[h])
        in_maps.append(m)
    return in_maps


def run(inputs, trace=False, **kw):
    nc = get_nc()
    in_maps = make_in_maps(**inputs)
    res = run_bass_kernel_spmd(nc, in_maps, core_ids=list(range(8)), trace=trace, **kw)
    ys = [np.asarray(res.results[i]["out"], dtype=np.float32) for i in range(8)]
    y = np.stack([ys[2 * n] + ys[2 * n + 1] for n in range(N)])
    return y.reshape(N, C, 64, 64), res


def kernel(**inputs):
    y, _ = run(inputs, trace=False)
    return y
